# revision 1
# baseline (speedup 1.0000x reference)
"""GAT layer (dense adjacency) on 8 Trainium2 NeuronCores.

Problem: H = elu(softmax_j(mask(A, leaky_relu(Wh1_i + Wh2_j))) @ Wh),
A: [8, 2048, 2048] 0/1 f32, X: [8, 2048, 64], Ws: [64, 64], a: [128, 1].

Sharding: data-parallel over batch B=8 -> one batch element per core.

Single pass over 16 j-slabs (A column tiles), all 2048 i-columns per slab,
paced by the A DMA (~2.9 us per 1 MiB slab at 360 GB/s):
  - Host precomputes Wh = X@Ws, Wh1 = Wh@a1, Wh2 = Wh@a2 (tiny: 0.1% of
    work) and packs the small inputs into three DMA blobs (small consts
    first so the first slab's compute starts early).
  - Per slab the masked logits are built in PSUM (4 single-bank chunks of
    512 i-cols, ring of 5 banks) by the tensor engine:
        pp[j, i] = ones2^T @ [Wh1_hi; Wh1_lo]  (broadcast of Wh1 along j)
                 + (A_block)^T @ (C*I)         (mask: C=512 where edge, 0 else)
    (the matmul with C*I transposes A; no elementwise mask multiply needed).
  - LeakyRelu(pp + (Wh2[j]-C)) split ACT/DVE per slab: ACT Prelu for 672
    cols; DVE z1 = pp+bias (fp16), z2 = 0.2*z1 (4x mode), max(z1,z2)
    (2x mode) for the other 1376 cols.  ACT Exp(e - S) -> pa fp16.
  - H accumulates in natural [i, d] layout: per i-tile m,
    acc[:, col(m):col(m)+65] += pa[:, 128m:128(m+1)]^T @ WhAug  (fp16
    inputs, ones column gives row sums).  The accumulator is zeroed once
    and all matmuls use start=False (start=True resets the whole PSUM
    bank, which would wipe co-resident regions).  3 banks, 7/7/2 packing.
  - The last slab splits its z2/max/Exp into three pa pieces aligned with
    the accumulator banks, so each bank's epilogue chain starts as soon
    as its own data is complete.
  - Epilogue per bank (batched, no per-tile ops): strided reciprocals ->
    rc; y = acc * broadcast(rc) (one TT); elu via the identity
    elu(y) = max(min(e^y - 1, 0), y):  W = Exp(y), t1 = min(W-1,0) (TSP),
    H = max(t1, y) (TT, fp16), one output DMA per bank.  H is written
    fp16 and cast to f32 on the host (5e-4 quantization, halves the
    output DMA).
"""
import sys

for _p in ("/opt/trn_rl_repo",):
    if _p not in sys.path:
        sys.path.append(_p)

import numpy as np
import ml_dtypes

import concourse.bass as bass
import concourse.bacc as bacc
import concourse.tile as tile
from concourse import mybir
from concourse import bass_utils

F32 = mybir.dt.float32
BF16 = mybir.dt.bfloat16
FP16 = mybir.dt.float16
AF = mybir.ActivationFunctionType
ALU = mybir.AluOpType

B, N, F, D = 8, 2048, 64, 64
NT = N // 128          # 16 j-slabs / i-tiles
C_MASK = 512.0
ALPHA = 0.2
P_ACT = 640            # i-cols whose LeakyRelu runs on ACT (Prelu)
L_DVE = 2048 - P_ACT   # i-cols on the DVE z-chain (1376)
# blobS f32 column layout: biasT | negS | pad | ci(bf16)
BS_BIAS, BS_NEGS, BS_CI, BS_W = 0, 16, 18, 82

_CACHED = {}


def _acc_col(m):
    """Column offset of i-tile m inside the [128, 1536] PSUM accumulator
    (7 tiles in bank 0, 7 in bank 1, 2 in bank 2)."""
    return 512 * (m // 7) + 65 * (m % 7)


def _build_program():
    nc = bacc.Bacc("TRN2", target_bir_lowering=False, debug=False)

    A_d = nc.dram_tensor("A", [N, N], F32, kind="ExternalInput")
    blobS_d = nc.dram_tensor("blobS", [128, BS_W], F32, kind="ExternalInput")
    blobW_d = nc.dram_tensor("blobW", [128, 520], F32, kind="ExternalInput")
    blob2_d = nc.dram_tensor("blob2", [2, 2176], BF16, kind="ExternalInput")
    H_d = nc.dram_tensor("H", [N, D], FP16, kind="ExternalOutput")

    with tile.TileContext(nc) as tc:
        with tc.tile_pool(name="const", bufs=1) as cp, \
             tc.tile_pool(name="aslab", bufs=8) as ap_pool, \
             tc.tile_pool(name="work", bufs=3) as wp, \
             tc.tile_pool(name="outp", bufs=1) as op_pool, \
             tc.tile_pool(name="psP", bufs=5, space="PSUM") as psP, \
             tc.tile_pool(name="psA", bufs=1, space="PSUM") as psA:

            # ---- constants (3 DMAs; small ones first) ----
            cbS = cp.tile([128, BS_W], F32, name="cbS")
            nc.sync.dma_start(cbS[:], blobS_d.ap())
            cb2 = cp.tile([2, 2176], BF16, name="cb2")
            nc.sync.dma_start(cb2[:], blob2_d.ap())
            cbW = cp.tile([128, 520], F32, name="cbW")
            biasT = cbS[:, BS_BIAS:BS_BIAS + NT]
            negS = cbS[:, BS_NEGS:BS_NEGS + 1]
            ci = cbS[:].bitcast(BF16)[:, 2 * BS_CI:2 * BS_CI + 128]
            wh1p = cb2[:, 0:2048]
            ones2 = cb2[:, 2048:2176]
            whaug = cbW[:].bitcast(FP16)[:, 0:1040]          # [128, 16*65]
            alpha02 = cp.tile([128, 1], F32, name="alpha02")
            nc.vector.memset(alpha02[:], ALPHA)
            # preload the ACT table set during input DMA
            warm = cp.tile([1, 1], F32, name="warm")
            nc.vector.memset(warm[:], 0.0)
            warm2 = cp.tile([1, 1], F32, name="warm2")
            nc.scalar.activation(warm2[:], warm[:], AF.Exp, bias=0.0, scale=1.0)

            # H^pre accumulator (+ row sums), [i, d] layout, 3 banks.
            # Zeroed once; the accumulating matmuls all use start=False
            # (start=True resets the whole PSUM bank, wiping the slab-0
            # contribution of co-resident regions).
            # one tile per bank so each bank's epilogue chain only waits on
            # its own writers (dep tracking is tile-granular)
            accs = [psA.tile([128, 512], F32, name=f"acc{b3}")
                    for b3 in range(3)]
            for b3 in range(3):
                nc.vector.memset(accs[b3][:], 0.0)

            aslabs = {}
            state = {}

            def dma_slab(t, split=False):
                sl = ap_pool.tile([128, N], F32, name=f"aslab{t}", tag="aslab")
                # sl[p, 128*r + q] = A[128*r + p, 128*t + q]
                if split:  # 4 row-block chunks so fills can start early
                    for rr in range(4):
                        nc.sync.dma_start(
                            sl[:, 512 * rr:512 * (rr + 1)]
                            .rearrange("p (r q) -> p r q", q=128),
                            A_d.ap()[512 * rr:512 * (rr + 1),
                                     128 * t:128 * (t + 1)]
                            .rearrange("(r p) q -> p r q", p=128),
                        )
                else:
                    nc.sync.dma_start(
                        sl[:].rearrange("p (r q) -> p r q", q=128),
                        A_d.ap()[:, 128 * t:128 * (t + 1)]
                        .rearrange("(r p) q -> p r q", p=128),
                    )
                aslabs[t] = sl

            def fills(t):
                # 4 single-bank chunks of 512 i-cols each
                chunks = []
                for c in range(4):
                    pp = psP.tile([128, 512], F32, name=f"pp{c}_{t}", tag="pp")
                    nc.tensor.matmul(
                        pp[:], ones2[:], wh1p[:, 512 * c:512 * (c + 1)],
                        start=True, stop=False,
                    )
                    for k4 in range(4):
                        r = 4 * c + k4
                        nc.tensor.matmul(
                            pp[:, 128 * k4:128 * (k4 + 1)],
                            aslabs[t][:].bitcast(BF16)[:, 256 * r + 1:256 * (r + 1):2],
                            ci[:],
                            start=False, stop=True, skip_group_check=True,
                        )
                    chunks.append(pp)
                return chunks

            def prelu_z1(t, pp):
                # ACT Prelu for cols [0, 672); DVE z1 (fp16) for the rest
                e_act = wp.tile([128, P_ACT], F32, name=f"ea{t}", tag="ea")
                zt = wp.tile([128, L_DVE], FP16, name=f"zt{t}", tag="zt")
                b_ap = biasT[:, t:t + 1]
                nc.scalar.activation(
                    e_act[:, 0:512], pp[0][:], AF.Prelu,
                    bias=b_ap, scale=1.0, alpha=alpha02[:])
                nc.scalar.activation(
                    e_act[:, 512:P_ACT], pp[1][:, 0:P_ACT - 512], AF.Prelu,
                    bias=b_ap, scale=1.0, alpha=alpha02[:])
                nc.vector.tensor_scalar(
                    zt[:, 0:384], pp[1][:, 128:512], b_ap, None, ALU.add)
                nc.vector.tensor_scalar(
                    zt[:, 384:896], pp[2][:], b_ap, None, ALU.add)
                nc.vector.tensor_scalar(
                    zt[:, 896:1408], pp[3][:], b_ap, None, ALU.add)
                state[t] = {"ea": e_act, "zt": zt}

            def z2_tt(t, parts):
                # parts: list of (start, end) ranges within zt
                st = state[t]
                st["ed"] = []
                for (s0, s1) in parts:
                    one = len(parts) == 1
                    z2t = wp.tile([128, s1 - s0], FP16, name=f"z2t{t}_{s0}",
                                  tag=f"z2s{s0}_{s1}", bufs=3)
                    e_dve = wp.tile([128, s1 - s0], FP16, name=f"ed{t}_{s0}",
                                    tag=f"eds{s0}_{s1}", bufs=3)
                    nc.vector.tensor_scalar(
                        z2t[:], st["zt"][:, s0:s1], ALPHA, None, ALU.mult)
                    nc.vector.tensor_tensor(
                        e_dve[:], st["zt"][:, s0:s1], z2t[:], ALU.max)
                    st["ed"].append((s0, s1, e_dve))

            def exp_act(t, pa_map):
                # pa_map: list of (col0, col1, tile); ExpA writes [0, P_ACT)
                st = state[t]
                st["pa_map"] = pa_map
                c0, c1, tl = pa_map[0]
                assert c0 == 0 and c1 >= P_ACT
                nc.scalar.activation(
                    tl[:, 0:P_ACT], st["ea"][:], AF.Exp, bias=negS, scale=1.0)

            def _pa_slice(pa_map, c0, c1):
                for (p0, p1, tl) in pa_map:
                    if p0 <= c0 and c1 <= p1:
                        return tl[:, c0 - p0:c1 - p0]
                raise AssertionError((c0, c1))

            def exp_dve(t):
                st = state[t]
                for (s0, s1, e_dve) in st["ed"]:
                    tgt = _pa_slice(st["pa_map"], P_ACT + s0, P_ACT + s1)
                    nc.scalar.activation(
                        tgt, e_dve[:], AF.Exp, bias=negS, scale=1.0)

            def accum(t):
                pa_map = state.pop(t)["pa_map"]
                for m in range(NT):
                    c0 = 65 * (m % 7)
                    nc.tensor.matmul(
                        accs[m // 7][:, c0:c0 + D + 1],
                        _pa_slice(pa_map, 128 * m, 128 * (m + 1)),
                        whaug[:, (D + 1) * t:(D + 1) * (t + 1)],
                        start=False, stop=(t == NT - 1),
                        skip_group_check=True,
                    )

            # ---- software-pipelined main loop ----
            # Lag structure keeps every engine queue head (nearly) always
            # ready: DVE runs lag-1 z2/max; ACT runs lag-1/lag-2 Exps before
            # this slab's Prelu; PE runs fills before the lag-3 accumulation.
            # The last slab (t=15) splits into three pa pieces aligned with
            # the accumulator banks (tiles 0-7 / 8-13 / 14-15) so the
            # per-bank epilogue chains start as early as possible.
            LAST = NT - 1
            dma_slab(0, split=True)
            whaug_dma_done = False
            for t in range(1, 4):
                dma_slab(t)
            nc.sync.dma_start(cbW[:], blobW_d.ap())
            for k in range(NT + 3):
                if k + 4 < NT:
                    dma_slab(k + 4)
                if 1 <= k <= NT:
                    t = k - 1
                    if t == LAST:
                        z2_tt(t, [(0, 384), (384, 1152), (1152, L_DVE)])
                        pa_map = [
                            (0, 1024, wp.tile([128, 1024], FP16,
                                              name="paL0", bufs=1)),
                            (1024, 1792, wp.tile([128, 768], FP16,
                                                 name="paL1", bufs=1)),
                            (1792, 2048, wp.tile([128, 256], FP16,
                                                 name="paL2", bufs=1)),
                        ]
                    else:
                        z2_tt(t, [(0, 384), (384, L_DVE)])
                        pa_map = [(0, 2048, wp.tile([128, 2048], FP16,
                                                    name=f"pa{t}", tag="pa",
                                                    bufs=4))]
                    exp_act(t, pa_map)
                if 2 <= k <= NT + 1:
                    exp_dve(k - 2)
                if k < NT:
                    pp = fills(k)
                    prelu_z1(k, pp)
                if k >= 3:
                    accum(k - 3)

            # ---- epilogue per accumulator bank: H = max(min(e^y-1, 0), y),
            # y = H_pre * (1/s) ----
            banks = [(0, 7), (7, 7), (14, 2)]  # (first tile, count)
            for bi, (m0, g) in enumerate(banks):
                A = accs[bi]
                rc_b = op_pool.tile([128, g], F32, name=f"rc{bi}")
                y_b = op_pool.tile([128, g * 64], FP16, name=f"y{bi}")
                W_b = op_pool.tile([128, g * 64], FP16, name=f"W{bi}")
                t1_b = op_pool.tile([128, g * 64], FP16, name=f"t1{bi}")
                h_b = op_pool.tile([128, g * 64], FP16, name=f"h{bi}")
                nc.vector.reciprocal(
                    rc_b[:], A[:, 64:64 + (g - 1) * 65 + 1:65])
                hp = A[:, 0:g * 65] \
                    .rearrange("p (g c) -> p g c", c=65)[:, :, 0:64]
                rcb = rc_b[:].unsqueeze(2).broadcast_to([128, g, 64])
                nc.vector.tensor_tensor(
                    y_b[:].rearrange("p (g c) -> p g c", c=64),
                    hp, rcb, ALU.mult)
                nc.scalar.activation(
                    W_b[:], y_b[:], AF.Exp, bias=0.0, scale=1.0)
                nc.vector.tensor_scalar(
                    t1_b[:], W_b[:], -1.0, 0.0, ALU.add, ALU.min)
                nc.vector.tensor_tensor(
                    h_b[:], t1_b[:], y_b[:], ALU.max)
                nc.sync.dma_start(
                    H_d.ap()[128 * m0:128 * (m0 + g), :]
                    .rearrange("(t p) d -> p t d", p=128),
                    h_b[:].rearrange("p (t d) -> p t d", d=D),
                )

    nc.compile()
    return nc


def _get_program():
    if "nc" not in _CACHED:
        _CACHED["nc"] = _build_program()
    return _CACHED["nc"]


def _host_prep(A, X, Ws, a):
    """Per-core host-side input preparation (cheap: ~67 MFLOP total)."""
    f64 = np.float64
    in_maps = []
    ci = (C_MASK * np.eye(128)).astype(ml_dtypes.bfloat16)
    for b in range(B):
        Wh = X[b].astype(f64) @ Ws.astype(f64)            # [N, D]
        Wh1 = (Wh @ a[:D].astype(f64))[:, 0]              # [N]
        Wh2 = (Wh @ a[D:].astype(f64))[:, 0]              # [N]
        S = max(0.0, float(Wh1.max() + Wh2.max()) - 10.5)
        whaug = np.ones((N, D + 1), np.float16)
        whaug[:, :D] = Wh.astype(np.float16)
        wh1_hi = Wh1.astype(ml_dtypes.bfloat16)
        wh1_lo = (Wh1 - wh1_hi.astype(f64)).astype(ml_dtypes.bfloat16)
        biasT = (Wh2 - C_MASK).astype(np.float32).reshape(NT, 128).T

        blobS = np.zeros((128, BS_W), np.float32)
        blobS[:, BS_BIAS:BS_BIAS + NT] = biasT
        blobS[:, BS_NEGS] = -S
        blobS.view(np.uint16)[:, 2 * BS_CI:2 * BS_CI + 128] = ci.view(np.uint16)

        blobW = np.zeros((128, 520), np.float32)
        blobW.view(np.uint16)[:, 0:1040] = whaug.reshape(NT, 128, D + 1) \
            .transpose(1, 0, 2).reshape(128, 1040).view(np.uint16)

        blob2 = np.zeros((2, 2176), ml_dtypes.bfloat16)
        blob2[0, 0:2048] = wh1_hi
        blob2[1, 0:2048] = wh1_lo
        blob2[:, 2048:2176] = np.ones((2, 128), ml_dtypes.bfloat16)

        in_maps.append({
            "A": np.ascontiguousarray(A[b]),
            "blobS": blobS,
            "blobW": blobW,
            "blob2": blob2,
        })
    return in_maps


def kernel(A, X, Ws, a, _trace=False, _trace_kwargs=None):
    A = np.asarray(A, np.float32)
    X = np.asarray(X, np.float32)
    Ws = np.asarray(Ws, np.float32)
    a = np.asarray(a, np.float32)
    nc = _get_program()
    in_maps = _host_prep(A, X, Ws, a)
    kw = {}
    if _trace:
        kw = {"trace": True, **(_trace_kwargs or {})}
    res = bass_utils.run_bass_kernel_spmd(nc, in_maps, core_ids=list(range(B)), **kw)
    H = np.stack([np.asarray(res.results[b]["H"], np.float32) for b in range(B)])
    if _trace:
        kernel.last_results = res
    return H

